# revision 21
# baseline (speedup 1.0000x reference)
"""Trainium2 Bass kernel for the DLGN kernel-machine problem.

Reference computation (fp32):
    ig = inp @ gating[0]; dg = data @ gating[0]
    K  = sig(B*ig) @ sig(B*dg).T
    for l in 1..3:
        ig = ig @ gating[l]; dg = dg @ gating[l]
        K *= (sig(B*ig) @ sig(B*dg).T) / 512
    out = K @ alphas                      # [n_inp]

Strategy (8 NeuronCores, 2x4 shard: inp rows in 2 groups, data rows in 4;
host sums the 4 partials per i-block):
  - FLATTENED GATE CHAIN: host precomputes cumulative weight products
    W~_l = W_1...W_l (fp64), so layer l's pre-activation is x0 @ W~_l
    directly from the original input: no on-device chain, no PSUM->SBUF
    chain copies, no sequential layer dependency.
  - fp8 DoubleRow gate matmuls with hi/lo split inputs (x ~ xh + xl,
    e4m3 pair ~14-bit effective; W~ single fp8 at 32x prescale). Per
    (m-chunk, 512-col block): 4 DR matmuls (2 contraction halves x
    {xh, xl}). The kernel is PE *instruction-issue* bound (~240-290 ns
    per matmul on HW: ~107 compute + ~107 LD_WEIGHTS, not overlapped +
    issue), so the fp8-DR gates beat bf16 k-chunk gates on ldweights
    time at the same instruction count (512).
  - Asymmetric-centering fp8 trick: s = sig(4x) i-side, t = tanh(2x)
    d-side; 2*K_l = Si + s_i8 . t_d8 with Si = rowsum(s_i8) EXACT via
    fp8-DR matmul vs ones into PSUM. All matmuls fp8 DoubleRow: no
    dtype mode switches anywhere.
  - Combine per (stripe, ic), split across engines exactly as the
    proven schedule: L0 DVE stt (kps+Si)*alphas; L1/L2 ACT
    Identity(kps+Si-bias) -> SBUF then Pool multiply; L3 DVE stt with
    accum_out row-reduce. alphas prescaled by 2^-31 on host.
  - Final metric ~1.6e-2 (gate 2e-2), measured on HW.
"""

import numpy as np

import concourse.tile as tile
from concourse import bacc, mybir
from concourse.bass_utils import run_bass_kernel_spmd

BETA = 4.0
WIDTH = 512
DEPTH = 4
DIM = 512
N_I = 4096
N_D = 8192
R, C = 2, 4
NI_SH = N_I // R  # 2048
ND_SH = N_D // C  # 2048
D_STRIPE = 512
N_STRIPES = ND_SH // D_STRIPE  # 4
I_CHUNKS = NI_SH // 128  # 16
NB = NI_SH // 512  # 4 i-side column blocks
KC = DIM // 128  # 4
C_SCALE = (0.5**DEPTH) / float(WIDTH ** (DEPTH - 1))  # 2^-31, exact
WSC = 32.0  # weight prescale for fp8
XSC = 16.0  # input prescale for fp8
SIG_SCALE = BETA / (WSC * XSC)  # sig(4x) = sig(psum * 2^-7)
TANH_SCALE = (BETA / 2) / (WSC * XSC)  # tanh(2x) = tanh(psum * 2^-8)

F32 = mybir.dt.float32
FP8 = mybir.dt.float8e4
AFT = mybir.ActivationFunctionType
MULT = mybir.AluOpType.mult
ADD = mybir.AluOpType.add
DRM = mybir.MatmulPerfMode.DoubleRow

_NC = None


def _gate_layer(nc, gpsum, wq, x8, out8, aft, scale, l, nblocks, blk0=0):
    """One flattened gate layer over `nblocks` 512-col blocks: per
    (m-chunk, block) one PSUM tile fed by 4 fp8-DR matmuls
    (2 contraction halves x hi/lo input), then one activation to fp8.

    x8 layout: [128, blk, xv(2), h(2), j(2), 512]; out8 column block
    index within the tile is blk0+b."""
    for m in range(KC):
        for b in range(nblocks):
            ps = gpsum.tile([128, 512], F32, tag="gps")
            n = 0
            for h in range(2):
                for xv in range(2):
                    nc.tensor.matmul(
                        ps[:],
                        wq[:, l, h, :, m * 128 : (m + 1) * 128],
                        x8[:, blk0 + b, xv, h, :, :],
                        start=(n == 0),
                        stop=(n == 3),
                        perf_mode=DRM,
                    )
                    n += 1
            sl = slice(b * 512, (b + 1) * 512)
            nc.scalar.activation(out8[:, l, m, sl], ps[:], aft, scale=scale)


def _k_combine(nc, kpsum, kpool, upool, scrpool, ti8, td8, tisb, alp, y_acc, ic):
    """K-product + combine for one (stripe, i-chunk): per layer 2 fp8
    DoubleRow matmuls, then the running product split across engines."""
    isl = slice(ic * 128, (ic + 1) * 128)
    kblk = kpool.tile([128, D_STRIPE], F32, tag="kblk")
    for l in range(DEPTH):
        kps = kpsum.tile([128, 512], F32, tag="kps")
        for h in range(2):
            nc.tensor.matmul(
                kps[:],
                ti8[:, l, 2 * h : 2 * h + 2, isl],
                td8[:, l, 2 * h : 2 * h + 2, :],
                start=(h == 0),
                stop=(h == 1),
                perf_mode=DRM,
            )
        ti_ap = tisb[:, l, ic : ic + 1]
        if l == 0:
            nc.vector.scalar_tensor_tensor(kblk[:], kps[:], ti_ap, alp[:], ADD, MULT)
        elif l < DEPTH - 1:
            u = upool.tile([128, D_STRIPE], F32, tag="u")
            nc.scalar.activation(u[:], kps[:], AFT.Identity, bias=ti_ap)
            nc.gpsimd.tensor_mul(kblk[:], u[:], kblk[:])
        else:
            part = scrpool.tile([128, 1], F32, tag="part")
            nc.vector.scalar_tensor_tensor(
                kblk[:], kps[:], ti_ap, kblk[:], ADD, MULT, accum_out=part[:, 0:1]
            )
            nc.vector.tensor_add(
                y_acc[:, ic : ic + 1], y_acc[:, ic : ic + 1], part[:, 0:1]
            )


def _build(repeat=1):
    nc = bacc.Bacc("TRN2", target_bir_lowering=False, debug=False, num_devices=8)

    wq_d = nc.dram_tensor("wq", [128, DEPTH, 2, 2, DIM], FP8, kind="ExternalInput")
    xi_d = nc.dram_tensor("xi", [128, NB, 2, 2, 2, 512], FP8, kind="ExternalInput")
    xd_d = nc.dram_tensor(
        "xd", [128, N_STRIPES, 2, 2, 2, 512], FP8, kind="ExternalInput"
    )
    alphas_d = nc.dram_tensor("alphas_s", [128, ND_SH], F32, kind="ExternalInput")
    y_d = nc.dram_tensor("y", [128, I_CHUNKS], F32, kind="ExternalOutput")

    from contextlib import nullcontext

    with tile.TileContext(nc) as tc:
        with (
            tc.tile_pool(name="w", bufs=1) as wpool,
            tc.tile_pool(name="x", bufs=1) as xpool,
            tc.tile_pool(name="sigi", bufs=1) as sigi_pool,
            tc.tile_pool(name="yp", bufs=1) as ypool,
            tc.tile_pool(name="cst", bufs=1) as cpool,
            tc.tile_pool(name="gpsum", bufs=3, space="PSUM") as gpsum,
            tc.tile_pool(name="kpsum", bufs=4, space="PSUM") as kpsum,
            tc.tile_pool(name="tip", bufs=1, space="PSUM") as tippool,
            tc.For_i(0, repeat, 1) if repeat > 1 else nullcontext(),
        ):
            wq = wpool.tile([128, DEPTH, 2, 2, DIM], FP8)
            for l in range(DEPTH):
                nc.sync.dma_start(wq[:, l], wq_d.ap()[:, l])
            xi = xpool.tile([128, NB, 2, 2, 2, 512], FP8, name="xi")
            for b in range(NB):
                nc.sync.dma_start(xi[:, b], xi_d.ap()[:, b])

            ones8_t = cpool.tile([128, 2, 16], FP8)
            nc.gpsimd.memset(ones8_t[:], 1.0)
            ones8 = ones8_t[:, :, 0:1]

            ti8 = sigi_pool.tile([128, DEPTH, KC, NI_SH], FP8)
            y_acc = ypool.tile([128, I_CHUNKS], F32)
            nc.gpsimd.memset(y_acc[:], 0.0)

            tips = tippool.tile([128, DEPTH, I_CHUNKS], F32)
            tisb = ypool.tile([128, DEPTH, I_CHUNKS], F32, name="tisb")

            # ---- Phase A: i-side gates (sigmoid -> fp8) + Si row-sums ----
            for l in range(DEPTH):
                _gate_layer(nc, gpsum, wq, xi, ti8, AFT.Sigmoid, SIG_SCALE, l, NB)
                for ic in range(I_CHUNKS):
                    isl = slice(ic * 128, (ic + 1) * 128)
                    for h in range(2):
                        nc.tensor.matmul(
                            tips[:, l, ic : ic + 1],
                            ti8[:, l, 2 * h : 2 * h + 2, isl],
                            ones8,
                            start=(h == 0),
                            stop=(h == 1),
                            perf_mode=DRM,
                        )

            nc.vector.tensor_copy(tisb[:], tips[:])

            # ---- Phase B: d-stripes (tanh -> fp8) + K products + combine ----
            with (
                tc.tile_pool(name="sigd", bufs=2) as sigd_pool,
                tc.tile_pool(name="alp", bufs=2) as alpool,
                tc.tile_pool(name="kblk", bufs=4) as kpool,
                tc.tile_pool(name="u", bufs=6) as upool,
                tc.tile_pool(name="scr", bufs=4) as scrpool,
            ):
                xd = xpool.tile([128, N_STRIPES, 2, 2, 2, 512], FP8, name="xd")
                for s in range(N_STRIPES):
                    nc.sync.dma_start(xd[:, s], xd_d.ap()[:, s])
                for s in range(N_STRIPES):
                    ssl = slice(s * D_STRIPE, (s + 1) * D_STRIPE)
                    alp = alpool.tile([128, D_STRIPE], F32, tag="alp")
                    nc.sync.dma_start(alp[:], alphas_d.ap()[:, ssl])

                    td8 = sigd_pool.tile([128, DEPTH, KC, D_STRIPE], FP8, tag="td8")
                    for l in range(DEPTH):
                        _gate_layer(
                            nc, gpsum, wq, xd, td8, AFT.Tanh, TANH_SCALE, l, 1, blk0=s
                        )

                    for ic in range(I_CHUNKS):
                        _k_combine(
                            nc, kpsum, kpool, upool, scrpool,
                            ti8, td8, tisb, alp, y_acc, ic,
                        )

            nc.sync.dma_start(y_d.ap(), y_acc[:])

    nc.compile()
    return nc


def _get_nc():
    global _NC
    if _NC is None:
        _NC = _build()
    return _NC


def _hilo(x):
    import ml_dtypes

    f8 = ml_dtypes.float8_e4m3
    hi = x.astype(f8)
    lo = (x - hi.astype(np.float32)).astype(f8)
    return hi, lo


def _dr_pack(mat):
    """[512, n] -> [128, 2(h), 2(j), n] with row w = h*256 + j*128 + p."""
    n = mat.shape[1]
    return np.ascontiguousarray(mat.reshape(2, 2, 128, n).transpose(2, 0, 1, 3))


def make_in_maps(inp, data, gating, alphas):
    import ml_dtypes

    f8 = ml_dtypes.float8_e4m3
    inp = np.asarray(inp, dtype=np.float32)
    data = np.asarray(data, dtype=np.float32)
    gating = np.asarray(gating, dtype=np.float64)
    alphas = np.asarray(alphas, dtype=np.float32) * np.float32(C_SCALE)

    # Cumulative weight products (flattened gate chain), single fp8
    wt = gating[0]
    wq = np.empty((128, DEPTH, 2, 2, DIM), dtype=f8)
    for l in range(DEPTH):
        if l > 0:
            wt = wt @ gating[l]
        wq[:, l] = _dr_pack((WSC * wt).astype(np.float32).astype(f8).astype(np.float32))

    def pack_x(xT):  # [512, cols] fp32 -> [128, nblk, 2, 2, 2, 512] fp8
        nblk = xT.shape[1] // 512
        xh, xl = _hilo(XSC * xT)
        out = np.empty((128, nblk, 2, 2, 2, 512), dtype=f8)
        for b in range(nblk):
            cs = slice(b * 512, (b + 1) * 512)
            out[:, b, 0] = _dr_pack(xh[:, cs])
            out[:, b, 1] = _dr_pack(xl[:, cs])
        return out

    in_maps = []
    for r in range(R):
        xi = pack_x(np.ascontiguousarray(inp[r * NI_SH : (r + 1) * NI_SH].T))
        for c in range(C):
            xd = pack_x(np.ascontiguousarray(data[c * ND_SH : (c + 1) * ND_SH].T))
            al = np.ascontiguousarray(
                np.broadcast_to(alphas[c * ND_SH : (c + 1) * ND_SH], (128, ND_SH))
            )
            in_maps.append({"wq": wq, "xi": xi, "xd": xd, "alphas_s": al})
    return in_maps


def kernel(inp, data, gating, alphas):
    nc = _get_nc()
    in_maps = make_in_maps(inp, data, gating, alphas)
    res = run_bass_kernel_spmd(nc, in_maps, core_ids=list(range(R * C))).results

    y = np.empty(N_I, dtype=np.float32)
    for r in range(R):
        acc = res[r * C]["y"].T.reshape(NI_SH).copy()
        for c in range(1, C):
            acc += res[r * C + c]["y"].T.reshape(NI_SH)
        y[r * NI_SH : (r + 1) * NI_SH] = acc
    return y


# revision 22
# speedup vs baseline: 1.0136x; 1.0136x over previous
"""Trainium2 Bass kernel for the DLGN kernel-machine problem.

Reference computation (fp32):
    ig = inp @ gating[0]; dg = data @ gating[0]
    K  = sig(B*ig) @ sig(B*dg).T
    for l in 1..3:
        ig = ig @ gating[l]; dg = dg @ gating[l]
        K *= (sig(B*ig) @ sig(B*dg).T) / 512
    out = K @ alphas                      # [n_inp]

Shapes: inp [4096, 512], data [8192, 512], gating [4, 512, 512],
alphas [8192]; out [4096] fp32.

Strategy (8 NeuronCores):
  - 2D shard R=2 x C=4: inp rows in 2 groups of 2048, data rows in 4 groups
    of 2048. core = r*C + c computes y_partial[r-block] over its d-block;
    host sums C partials per i-block. R=2,C=4 minimizes replicated gate-chain
    work (4096/R + 8192/C columns).
  - Gate chains run in bf16 (inputs/weights/intermediates), contraction on
    partitions, activations fed transposed from host; zero on-device
    transposes. bf16 halves DMA traffic and weight-load time vs f32r; its
    rounding adds <1e-3 to the final metric.
  - Asymmetric-centering fp8 trick: with s = sig(4x) and t = tanh(2x)
    (so s_d = (1+t_d)/2), per layer
        2*K_l = Si + s_i8 . t_d8
    where Si = rowsum(s_i8) is EXACT (fp8 matmul vs ones, f32 PSUM accum)
    and rides the per-partition scalar/bias slot of the combine - no PSUM
    preload instructions at all. Centering the d-side routes the bulk of
    K_l through the exact Si path, cutting fp8 quantization error ~2x vs
    naive fp8 (measured 1.25e-2 final metric vs 2.4e-2; gate is 2e-2).
  - The big K-product matmuls run as fp8e4 DoubleRow (2 fp8 weights/PE
    cell, 256-contraction per instruction, ~2x rate): 2 instructions per
    (stripe, i-chunk, layer) with NO dtype mode switches anywhere in the
    hot loop (f32r instructions interleaved with fp8 cost ~750ns each,
    measured; all-fp8 avoids that entirely).
  - Si row-sums: s_i8^T @ ones8 DoubleRow -> PSUM [128,1] slots, copied
    once to SBUF after phase A.
  - Combine, split across three engines per (stripe, ic): layer 0 on DVE
    (stt: (kps+Si)*alphas), layers 1-2 via ACT Identity(kps + Si-bias) ->
    SBUF then Pool multiply (Pool cannot read PSUM), layer 3 on DVE with
    accum_out row-reduce into y. alphas pre-scaled on host by
    (1/2)^4 * 512^-3 = 2^-31 exact.
  - ACT runs Sigmoid/Tanh/Identity/Copy which all live in ONE activation
    table set (sigmoid_and_others) - single table load, no thrash.
"""

import numpy as np

import concourse.tile as tile
from concourse import bacc, mybir
from concourse.bass_utils import run_bass_kernel_spmd

BETA = 4.0
WIDTH = 512
DEPTH = 4
DIM = 512
N_I = 4096
N_D = 8192
R, C = 2, 4
NI_SH = N_I // R  # 2048
ND_SH = N_D // C  # 2048
D_STRIPE = 512
N_STRIPES = ND_SH // D_STRIPE  # 4
I_CHUNKS = NI_SH // 128  # 16
KC = DIM // 128  # 4 contraction chunks
C_SCALE = (0.5**DEPTH) / float(WIDTH ** (DEPTH - 1))  # 2^-31, exact

F32 = mybir.dt.float32
BF16 = mybir.dt.bfloat16
FP8 = mybir.dt.float8e4
AFT = mybir.ActivationFunctionType
MULT = mybir.AluOpType.mult
ADD = mybir.AluOpType.add
DRM = mybir.MatmulPerfMode.DoubleRow

_NC = None


def _gate_layer(nc, gpsum, W, prev, nxt, out8, aft, l, ncols):
    """One gate-chain layer over `ncols` columns: bf16 matmul chain +
    activation to fp8 + (for l<3) chain copy for the next layer."""
    for m in range(KC):
        for nb in range(ncols // 512):
            sl = slice(nb * 512, (nb + 1) * 512)
            ps = gpsum.tile([128, 512], F32, tag="gps")
            for k in range(KC):
                nc.tensor.matmul(
                    ps[:],
                    W[:, l, k, m * 128 : (m + 1) * 128],
                    prev[:, k, sl],
                    start=(k == 0),
                    stop=(k == KC - 1),
                )
            nc.scalar.activation(
                out8[:, l, m, sl],
                ps[:],
                aft,
                scale=BETA if aft == AFT.Sigmoid else BETA / 2,
            )
            if nxt is not None:
                nc.vector.tensor_copy(nxt[:, m, sl], ps[:])


def _k_combine(nc, kpsum, kpool, upool, scrpool, ti8, td8, tisb, alp, y_acc, ic):
    """K-product + combine for one (stripe, i-chunk): per layer 2 fp8
    DoubleRow matmuls, then the running product split across engines."""
    isl = slice(ic * 128, (ic + 1) * 128)
    kblk = kpool.tile([128, D_STRIPE], F32, tag="kblk")
    for l in range(DEPTH):
        kps = kpsum.tile([128, 512], F32, tag="kps")
        for h in range(2):
            nc.tensor.matmul(
                kps[:],
                ti8[:, l, 2 * h : 2 * h + 2, isl],
                td8[:, l, 2 * h : 2 * h + 2, :],
                start=(h == 0),
                stop=(h == 1),
                perf_mode=DRM,
            )
        ti_ap = tisb[:, l, ic : ic + 1]
        if l == 0:
            nc.vector.scalar_tensor_tensor(kblk[:], kps[:], ti_ap, alp[:], ADD, MULT)
        elif l < DEPTH - 1:
            u = upool.tile([128, D_STRIPE], F32, tag="u")
            nc.scalar.activation(u[:], kps[:], AFT.Identity, bias=ti_ap)
            nc.gpsimd.tensor_mul(kblk[:], u[:], kblk[:])
        else:
            part = scrpool.tile([128, 1], F32, tag="part")
            nc.vector.scalar_tensor_tensor(
                kblk[:], kps[:], ti_ap, kblk[:], ADD, MULT, accum_out=part[:, 0:1]
            )
            nc.vector.tensor_add(
                y_acc[:, ic : ic + 1], y_acc[:, ic : ic + 1], part[:, 0:1]
            )


def _build(repeat=1):
    nc = bacc.Bacc("TRN2", target_bir_lowering=False, debug=False, num_devices=8)

    inpT_d = nc.dram_tensor("inpT", [DIM, NI_SH], BF16, kind="ExternalInput")
    dataT_d = nc.dram_tensor("dataT", [DIM, ND_SH], BF16, kind="ExternalInput")
    gating_d = nc.dram_tensor("gating", [DEPTH, DIM, DIM], BF16, kind="ExternalInput")
    alphas_d = nc.dram_tensor("alphas_s", [128, ND_SH], F32, kind="ExternalInput")
    y_d = nc.dram_tensor("y", [128, I_CHUNKS], F32, kind="ExternalOutput")

    from contextlib import nullcontext

    with tile.TileContext(nc) as tc:
        with (
            tc.tile_pool(name="w", bufs=1) as wpool,
            tc.tile_pool(name="sigi", bufs=1) as sigi_pool,
            tc.tile_pool(name="yp", bufs=1) as ypool,
            tc.tile_pool(name="cst", bufs=1) as cpool,
            tc.tile_pool(name="gpsum", bufs=3, space="PSUM") as gpsum,
            tc.tile_pool(name="kpsum", bufs=4, space="PSUM") as kpsum,
            tc.tile_pool(name="tip", bufs=1, space="PSUM") as tippool,
            tc.For_i(0, repeat, 1) if repeat > 1 else nullcontext(),
        ):
            W = wpool.tile([128, DEPTH, KC, DIM], BF16)
            for l in range(DEPTH):
                nc.sync.dma_start(
                    W[:, l],
                    gating_d.ap()[l].rearrange("(k p) n -> p k n", p=128),
                )
            ones8_t = cpool.tile([128, 2, 16], FP8)
            nc.gpsimd.memset(ones8_t[:], 1.0)
            ones8 = ones8_t[:, :, 0:1]

            ti8 = sigi_pool.tile([128, DEPTH, KC, NI_SH], FP8)
            y_acc = ypool.tile([128, I_CHUNKS], F32)
            nc.gpsimd.memset(y_acc[:], 0.0)

            # Si row-sums: accumulate in PSUM, then copy once to SBUF
            tips = tippool.tile([128, DEPTH, I_CHUNKS], F32)
            tisb = ypool.tile([128, DEPTH, I_CHUNKS], F32, name="tisb")

            # ---- Phase A: i-side gate chain (sigmoid -> fp8) + Si ----
            with tc.tile_pool(name="ig", bufs=2) as igpool:
                prev = igpool.tile([128, KC, NI_SH], BF16, tag="ig")
                inpT_r = inpT_d.ap().rearrange("(k p) n -> p k n", p=128)
                for k in range(KC):
                    nc.sync.dma_start(prev[:, k], inpT_r[:, k])
                for l in range(DEPTH):
                    nxt = (
                        igpool.tile([128, KC, NI_SH], BF16, tag="ig", name=f"ig{l}")
                        if l < DEPTH - 1
                        else None
                    )
                    _gate_layer(nc, gpsum, W, prev, nxt, ti8, AFT.Sigmoid, l, NI_SH)
                    for ic in range(I_CHUNKS):
                        isl = slice(ic * 128, (ic + 1) * 128)
                        for h in range(2):
                            nc.tensor.matmul(
                                tips[:, l, ic : ic + 1],
                                ti8[:, l, 2 * h : 2 * h + 2, isl],
                                ones8,
                                start=(h == 0),
                                stop=(h == 1),
                                perf_mode=DRM,
                            )
                    prev = nxt

            nc.vector.tensor_copy(tisb[:], tips[:])

            # ---- Phase B: d-stripes (tanh -> fp8) ----
            with (
                tc.tile_pool(name="dat", bufs=3) as datpool,
                tc.tile_pool(name="dg", bufs=3) as dgpool,
                tc.tile_pool(name="sigd", bufs=2) as sigd_pool,
                tc.tile_pool(name="alp", bufs=2) as alpool,
                tc.tile_pool(name="kblk", bufs=4) as kpool,
                tc.tile_pool(name="u", bufs=6) as upool,
                tc.tile_pool(name="scr", bufs=4) as scrpool,
            ):
                for s in range(N_STRIPES):
                    ssl = slice(s * D_STRIPE, (s + 1) * D_STRIPE)
                    dat = datpool.tile([128, KC, D_STRIPE], BF16, tag="dat")
                    dat_r = dataT_d.ap()[:, ssl].rearrange("(k p) n -> p k n", p=128)
                    for k in range(KC):
                        nc.sync.dma_start(dat[:, k], dat_r[:, k])
                    alp = alpool.tile([128, D_STRIPE], F32, tag="alp")
                    nc.sync.dma_start(alp[:], alphas_d.ap()[:, ssl])

                    td8 = sigd_pool.tile([128, DEPTH, KC, D_STRIPE], FP8, tag="td8")

                    prevd = dat
                    for l in range(DEPTH):
                        nxtd = (
                            dgpool.tile(
                                [128, KC, D_STRIPE], BF16, tag="dg", name=f"dg{s}_{l}"
                            )
                            if l < DEPTH - 1
                            else None
                        )
                        _gate_layer(nc, gpsum, W, prevd, nxtd, td8, AFT.Tanh, l, D_STRIPE)
                        prevd = nxtd

                    for ic in range(I_CHUNKS):
                        _k_combine(
                            nc, kpsum, kpool, upool, scrpool,
                            ti8, td8, tisb, alp, y_acc, ic,
                        )

            nc.sync.dma_start(y_d.ap(), y_acc[:])

    nc.compile()
    return nc


def _get_nc():
    global _NC
    if _NC is None:
        _NC = _build()
    return _NC


def make_in_maps(inp, data, gating, alphas):
    import ml_dtypes

    bf = ml_dtypes.bfloat16
    inp = np.ascontiguousarray(np.asarray(inp, dtype=np.float32).astype(bf))
    data = np.ascontiguousarray(np.asarray(data, dtype=np.float32).astype(bf))
    gating = np.ascontiguousarray(np.asarray(gating, dtype=np.float32).astype(bf))
    alphas = np.asarray(alphas, dtype=np.float32) * np.float32(C_SCALE)

    in_maps = []
    for r in range(R):
        inpT = np.ascontiguousarray(inp[r * NI_SH : (r + 1) * NI_SH].T)
        for c in range(C):
            dataT = np.ascontiguousarray(data[c * ND_SH : (c + 1) * ND_SH].T)
            al = np.ascontiguousarray(
                np.broadcast_to(np.asarray(alphas[c * ND_SH : (c + 1) * ND_SH]), (128, ND_SH))
            )
            in_maps.append(
                {"inpT": inpT, "dataT": dataT, "gating": gating, "alphas_s": al}
            )
    return in_maps


def kernel(inp, data, gating, alphas):
    nc = _get_nc()
    in_maps = make_in_maps(inp, data, gating, alphas)
    res = run_bass_kernel_spmd(nc, in_maps, core_ids=list(range(R * C))).results

    y = np.empty(N_I, dtype=np.float32)
    for r in range(R):
        acc = res[r * C]["y"].T.reshape(NI_SH).copy()
        for c in range(1, C):
            acc += res[r * C + c]["y"].T.reshape(NI_SH)
        y[r * NI_SH : (r + 1) * NI_SH] = acc
    return y
